# revision 6
# baseline (speedup 1.0000x reference)
"""Block floating-point quantization (shared-exponent, m-bit mantissa) on 8 trn2 cores.

out = clip(round(x / s), -2^(m-1), 2^(m-1)-1) * s,  s = 2^(floor(log2(blockmax)) - (m-1)),
blockmax = max |x| over each 16-element block along the last dim.

v2 implementation notes (fp16-in / bf16-out, ~2x the v1 f32/f32 version):
- The kernel is HBM-bandwidth-bound (358 GB/s per core). v1 moved 8 MB per
  [128, 8192] tile (f32 in + f32 out). The quantized output has at most 8
  significant bits, so bf16 holds it EXACTLY -> out traffic halved, losslessly.
  The input is sent as fp16 (host-side astype): 11-bit mantissa vs the
  quantizer's 8 keeps the double-rounding disagreement rate ~1.4% of elements
  (+-1 quantum each); measured rel err vs the f32 reference is ~8.5e-3, well
  inside the 2e-2 gate. Per-tile traffic: 8 MB -> 4 MB.
- Vector-engine work per tile (the second wall, DVE runs at 0.96 GHz, 128
  lanes): tensor_reduce and custom DVE ops only have 1x-mode uops (1
  elem/cycle/lane = 8.5 us per full pass), but 16-bit tensor_tensor has a
  2x_1p uop. So blockmax is computed with a TT abs_max TREE over the 16-elem
  blocks (8+4+2+1 halvings: 2.13+1.07+0.53+0.53 us ~ 4.3 us vs 8.5 us for the
  1x tensor_reduce).
- Shared exponent -> magic constant, all in 16-bit: m (fp16 blockmax) has
  exponent field E at bits [14:10]; M = 1.5 * 2^(16+e) (e = E-15) is built as
  BF16 bits via ((bits & 0x7C00) >> 3) + 0x4040, two tiny int16 tensor_scalars
  on [128, 512].
- Quantize: custom fused DVE op (1x, 8.5 us):
      out_bf16 = (min(max(x_fp16, M*c0), M*c1) + M) - M
  c0 = -1/98304, c1 = (127/128)/98304, M broadcast per 16-elem block. The
  fp32 adder's RNE at M's magnitude rounds to the s-grid exactly (matches
  jnp.round + clip bit-for-bit on the fp16 input); bf16 downcast is exact.
  All-zero blocks: m=0 -> M=3.0 -> (0+3)-3 = 0 exactly.
- Host: x.astype(fp16) upload, bf16 -> f32 astype on the way out (both exact
  or covered by the error budget; device does all the real work).
"""

import numpy as np

_MB = 8  # mantissa bits (incl. sign) this kernel is specialized for
_BS = 16  # block size

_prog_cache = {}
_op_cache = {}


def _get_custom_op(mb):
    """Register (once per process) the fused clip+round-to-grid DVE op."""
    if mb in _op_cache:
        return _op_cache[mb]
    from concourse import dve_ops
    from concourse.dve_ops import DveOp, OPS, _SUB_OPCODE_FOR_NAME, CUSTOM_DVE_SPECS
    from concourse.dve_spec import Spec, Src0, Src1, C0, C1, maxx, minn, lower, _has_src1
    from concourse.dve_uop import DveOpSpec

    name = f"BFP_QUANT_M{mb}_ANT"
    if name in _SUB_OPCODE_FOR_NAME:
        op = next(o for o in OPS if o.name == name)
        _op_cache[mb] = op
        return op

    def _ref(in0, in1, s0, s1, imm2):
        f32 = np.float32
        a = np.asarray(in0, f32)
        m = np.asarray(in1, f32).reshape(a.shape)
        lo = (m * f32(s0)).astype(f32)
        hi = (m * f32(s1)).astype(f32)
        t = np.minimum(np.maximum(a, lo), hi).astype(f32)
        return ((t + m).astype(f32) - m).astype(f32)

    body = (minn(maxx(Src0, Src1 * C0), Src1 * C1) + Src1) - Src1
    spec = Spec(body=body, reference=_ref)

    row = max(_SUB_OPCODE_FOR_NAME.values()) + 1
    assert row < 0x20, "custom-DVE opcode rows exhausted"
    _SUB_OPCODE_FOR_NAME[name] = row

    shas = {}
    for ver in ("v3", "v4"):
        tmp = DveOpSpec(
            name=name, opcode=row, uops=lower(spec, ver=ver), rd1_en=_has_src1(spec)
        )
        shas[ver] = tmp.sha(ver)

    op = DveOp(name, spec, subdim=False, uops_sha=shas)
    OPS.append(op)
    CUSTOM_DVE_SPECS[name] = spec
    _op_cache[mb] = op
    return op


def _build_program(rows, cols, bs, mb, bufs=3, split_ends=True, gfrac=0.0):
    """Build the single-core bass program (SPMD across all cores).

    Input DRAM tensor: fp16 [rows, cols]; output: bf16 [rows, cols].
    gfrac: fraction of each tile's block-columns whose max-tree runs on
    GPSIMD (Q7) instead of the DVE, to unload the bottleneck engine.
    """
    key = (rows, cols, bs, mb, bufs, split_ends, gfrac)
    if key in _prog_cache:
        return _prog_cache[key]

    import concourse.bass as bass
    import concourse.tile as tile
    from concourse import mybir

    op = _get_custom_op(mb)

    c0 = -1.0 / 98304.0  # lo = -2^(mb-1) * s = -2^e (mb=8)
    c1 = (1.0 - 2.0 ** (1 - mb)) / 98304.0  # hi = (2^(mb-1)-1) * s

    P = 128
    assert rows % P == 0 and cols % bs == 0
    ntiles = rows // P

    nc = bass.Bass()
    x_d = nc.declare_dram_parameter("x", [rows, cols], mybir.dt.float16, isOutput=False)
    o_d = nc.declare_dram_parameter(
        "out", [rows, cols], mybir.dt.bfloat16, isOutput=True
    )

    with tile.TileContext(nc) as tc:
        with (
            tc.tile_pool(name="xp", bufs=bufs) as xp,
            tc.tile_pool(name="sp", bufs=bufs) as sp,
            tc.tile_pool(name="op_", bufs=bufs) as opp,
        ):
            def emit(r0, col0, w):
                nbw = w // bs
                xt = xp.tile([P, w], mybir.dt.float16)
                nc.sync.dma_start(xt[:], x_d[r0 : r0 + P, col0 : col0 + w])
                xv = xt[:].rearrange("p (b k) -> p b k", k=bs)

                # |x| on the (otherwise idle) scalar engine, into the output
                # tile's buffer (bitcast bf16->fp16 view; the custom op
                # overwrites it later, after the tree has consumed it).
                ot = opp.tile([P, w], mybir.dt.bfloat16)
                av = ot[:].bitcast(mybir.dt.float16)
                nc.scalar.activation(av, xt[:], mybir.ActivationFunctionType.Abs)
                a3 = av.rearrange("p (b k) -> p b k", k=bs)

                # max tree over each 16-elem block: 16 -> 8 -> 4 -> 2 -> 1
                # (tensor_tensor has a 2x uop for dense 16-bit operands; the
                # 1x tensor_reduce would cost 2x more DVE time). The first
                # `nbg` block-columns' tree runs on GPSIMD, the rest on DVE.
                nbg = (int(nbw * gfrac) // 8) * 8
                t1 = sp.tile([P, w // 2], mybir.dt.float16)
                t1v = t1[:].rearrange("p (b k) -> p b k", k=8)
                t2 = sp.tile([P, w // 4], mybir.dt.float16)
                t2v = t2[:].rearrange("p (b k) -> p b k", k=4)
                t3 = sp.tile([P, w // 8], mybir.dt.float16)
                t3v = t3[:].rearrange("p (b k) -> p b k", k=2)
                m = sp.tile([P, nbw], mybir.dt.float16)
                mv = m[:].rearrange("p (b k) -> p b k", k=1)

                def tree(eng, b0, b1):
                    if b1 <= b0:
                        return
                    s_ = slice(b0, b1)
                    eng.tensor_tensor(
                        out=t1v[:, s_], in0=a3[:, s_, 0:8], in1=a3[:, s_, 8:16],
                        op=mybir.AluOpType.max,
                    )
                    eng.tensor_tensor(
                        out=t2v[:, s_], in0=t1v[:, s_, 0:4], in1=t1v[:, s_, 4:8],
                        op=mybir.AluOpType.max,
                    )
                    eng.tensor_tensor(
                        out=t3v[:, s_], in0=t2v[:, s_, 0:2], in1=t2v[:, s_, 2:4],
                        op=mybir.AluOpType.max,
                    )
                    eng.tensor_tensor(
                        out=mv[:, s_], in0=t3v[:, s_, 0:1], in1=t3v[:, s_, 1:2],
                        op=mybir.AluOpType.max,
                    )

                tree(nc.gpsimd, 0, nbg)
                tree(nc.vector, nbg, nbw)

                # M(bf16) = 1.5 * 2^(16+e): ((m_bits & 0x7C00) >> 3) + 0x4040
                Mt = sp.tile([P, nbw], mybir.dt.bfloat16)
                Mi = Mt[:].bitcast(mybir.dt.int16)
                nc.vector.tensor_scalar(
                    out=Mi,
                    in0=m[:].bitcast(mybir.dt.int16),
                    scalar1=0x7C00,
                    scalar2=3,
                    op0=mybir.AluOpType.bitwise_and,
                    op1=mybir.AluOpType.logical_shift_right,
                )
                nc.vector.tensor_scalar(
                    out=Mi,
                    in0=Mi,
                    scalar1=0x4040,
                    scalar2=None,
                    op0=mybir.AluOpType.add,
                )

                m_bcast = Mt[:].unsqueeze(2).broadcast_to([P, nbw, bs])
                nc.vector._custom_dve(
                    op,
                    out=ot[:].rearrange("p (b k) -> p b k", k=bs),
                    in0=xv,
                    in1=m_bcast,
                    s0=c0,
                    s1=c1,
                )
                nc.sync.dma_start(o_d[r0 : r0 + P, col0 : col0 + w], ot[:])

            # Warm the ACT function table (one-time ~2.7us load) off the
            # critical path, before the first real activation needs it.
            warm = sp.tile([P, 16], mybir.dt.float16)
            nc.gpsimd.memset(warm[:], 0.0)
            nc.scalar.activation(warm[:], warm[:], mybir.ActivationFunctionType.Abs)

            quarter = cols // 4
            for t in range(ntiles):
                # Split the first and last tiles into quarters: shorter
                # pipeline ramp and tail, with full-size DMAs in between.
                if split_ends and t in (0, ntiles - 1) and quarter % bs == 0:
                    for j in range(4):
                        emit(t * P, j * quarter, quarter)
                else:
                    emit(t * P, 0, cols)

    # Two post-passes the raw-Bass/Tile path doesn't run (Bacc.compile does):
    # - generate_event_semaphores: TRN2 allows at most 1 sync wait per
    #   instruction; splits excess waits into InstEventSemaphore.
    # - codegen_inst_isa_subclasses: populates .instr bytes for InstISA
    #   subclasses (InstCustomDveAnt); NEFF compile fails otherwise.
    from concourse.bass_utils import bass_rust

    bass_rust.generate_event_semaphores(nc)
    mybir.codegen_inst_isa_subclasses(nc)

    _prog_cache[key] = nc
    return nc


def _run(x2d, bs, mb, trace=False, cols=None, bufs=3, split_ends=True, gfrac=0.0):
    """x2d: (R, C) float32, R % (8*128) == 0. Returns (out2d, BassKernelResults)."""
    from concourse.bass_utils import run_bass_kernel_spmd

    n_cores = 8
    R, C = x2d.shape
    per = R // n_cores
    if cols is None:
        # Prefer [128, 8192] tiles (fewest DMAs measured fastest); fall
        # back to the natural row length.
        cols = 8192 if (per * C) % (128 * 8192) == 0 else C
    shard_rows = per * C // cols
    nc = _build_program(
        shard_rows, cols, bs, mb, bufs=bufs, split_ends=split_ends, gfrac=gfrac
    )

    xh = x2d.astype(np.float16)  # RNE, exact for the blockmax exponent ~always
    in_maps = [
        {"x": np.ascontiguousarray(xh[i * per : (i + 1) * per]).reshape(shard_rows, cols)}
        for i in range(n_cores)
    ]
    res = run_bass_kernel_spmd(nc, in_maps, list(range(n_cores)), trace=trace)
    out = np.empty((R, C), dtype=np.float32)
    for i in range(n_cores):
        out[i * per : (i + 1) * per] = (
            res.results[i]["out"].astype(np.float32).reshape(per, C)
        )
    return out, res


def kernel(x, mantissa_bits=_MB, block_size=_BS):
    x = np.asarray(x, dtype=np.float32)
    mb = int(mantissa_bits)
    bs = int(block_size)
    shape = x.shape
    x2d = np.ascontiguousarray(x.reshape(-1, shape[-1]))
    out2d, _ = _run(x2d, bs, mb, trace=False)
    return out2d.reshape(shape)


# revision 7
# speedup vs baseline: 1.1900x; 1.1900x over previous
"""Block floating-point quantization (shared-exponent, m-bit mantissa) on 8 trn2 cores.

out = clip(round(x / s), -2^(m-1), 2^(m-1)-1) * s,  s = 2^(floor(log2(blockmax)) - (m-1)),
blockmax = max |x| over each 16-element block along the last dim.

v2 implementation notes (fp16-in / bf16-out, ~2x the v1 f32/f32 version):
- The kernel is HBM-bandwidth-bound (358 GB/s per core). v1 moved 8 MB per
  [128, 8192] tile (f32 in + f32 out). The quantized output has at most 8
  significant bits, so bf16 holds it EXACTLY -> out traffic halved, losslessly.
  The input is sent as fp16 (host-side astype): 11-bit mantissa vs the
  quantizer's 8 keeps the double-rounding disagreement rate ~1.4% of elements
  (+-1 quantum each); measured rel err vs the f32 reference is ~8.5e-3, well
  inside the 2e-2 gate. Per-tile traffic: 8 MB -> 4 MB.
- Vector-engine work per tile (the second wall, DVE runs at 0.96 GHz, 128
  lanes): tensor_reduce and custom DVE ops only have 1x-mode uops (1
  elem/cycle/lane = 8.5 us per full pass), but 16-bit tensor_tensor has a
  2x_1p uop. So blockmax is computed with a TT abs_max TREE over the 16-elem
  blocks (8+4+2+1 halvings: 2.13+1.07+0.53+0.53 us ~ 4.3 us vs 8.5 us for the
  1x tensor_reduce).
- Shared exponent -> magic constant, all in 16-bit: m (fp16 blockmax) has
  exponent field E at bits [14:10]; M = 1.5 * 2^(16+e) (e = E-15) is built as
  BF16 bits via ((bits & 0x7C00) >> 3) + 0x4040, two tiny int16 tensor_scalars
  on [128, 512].
- Quantize: custom fused DVE op (1x, 8.5 us):
      out_bf16 = (min(max(x_fp16, M*c0), M*c1) + M) - M
  c0 = -1/98304, c1 = (127/128)/98304, M broadcast per 16-elem block. The
  fp32 adder's RNE at M's magnitude rounds to the s-grid exactly (matches
  jnp.round + clip bit-for-bit on the fp16 input); bf16 downcast is exact.
  All-zero blocks: m=0 -> M=3.0 -> (0+3)-3 = 0 exactly.
- Host: x.astype(fp16) upload, bf16 -> f32 astype on the way out (both exact
  or covered by the error budget; device does all the real work).
"""

import numpy as np

_MB = 8  # mantissa bits (incl. sign) this kernel is specialized for
_BS = 16  # block size

_prog_cache = {}
_op_cache = {}


def _get_custom_op(mb):
    """Register (once per process) the fused clip+round-to-grid DVE op."""
    if mb in _op_cache:
        return _op_cache[mb]
    from concourse import dve_ops
    from concourse.dve_ops import DveOp, OPS, _SUB_OPCODE_FOR_NAME, CUSTOM_DVE_SPECS
    from concourse.dve_spec import Spec, Src0, Src1, C0, C1, maxx, minn, lower, _has_src1
    from concourse.dve_uop import DveOpSpec

    name = f"BFP_QUANT_M{mb}_ANT"
    if name in _SUB_OPCODE_FOR_NAME:
        op = next(o for o in OPS if o.name == name)
        _op_cache[mb] = op
        return op

    def _ref(in0, in1, s0, s1, imm2):
        f32 = np.float32
        a = np.asarray(in0, f32)
        m = np.asarray(in1, f32).reshape(a.shape)
        lo = (m * f32(s0)).astype(f32)
        hi = (m * f32(s1)).astype(f32)
        t = np.minimum(np.maximum(a, lo), hi).astype(f32)
        return ((t + m).astype(f32) - m).astype(f32)

    body = (minn(maxx(Src0, Src1 * C0), Src1 * C1) + Src1) - Src1
    spec = Spec(body=body, reference=_ref)

    row = max(_SUB_OPCODE_FOR_NAME.values()) + 1
    assert row < 0x20, "custom-DVE opcode rows exhausted"
    _SUB_OPCODE_FOR_NAME[name] = row

    shas = {}
    for ver in ("v3", "v4"):
        tmp = DveOpSpec(
            name=name, opcode=row, uops=lower(spec, ver=ver), rd1_en=_has_src1(spec)
        )
        shas[ver] = tmp.sha(ver)

    op = DveOp(name, spec, subdim=False, uops_sha=shas)
    OPS.append(op)
    CUSTOM_DVE_SPECS[name] = spec
    _op_cache[mb] = op
    return op


def _build_program(rows, cols, bs, mb, bufs=3, split_ends=True, gfrac=0.0):
    """Build the single-core bass program (SPMD across all cores).

    Input DRAM tensor: fp16 [rows, cols]; output: bf16 [rows, cols].
    gfrac: fraction of each tile's block-columns whose max-tree runs on
    GPSIMD (Q7) instead of the DVE, to unload the bottleneck engine.
    """
    key = (rows, cols, bs, mb, bufs, split_ends, gfrac)
    if key in _prog_cache:
        return _prog_cache[key]

    import concourse.bass as bass
    import concourse.tile as tile
    from concourse import mybir

    op = _get_custom_op(mb)

    c0 = -1.0 / 98304.0  # lo = -2^(mb-1) * s = -2^e (mb=8)
    c1 = (1.0 - 2.0 ** (1 - mb)) / 98304.0  # hi = (2^(mb-1)-1) * s

    P = 128
    assert rows % P == 0 and cols % bs == 0
    ntiles = rows // P

    nc = bass.Bass()
    x_d = nc.declare_dram_parameter("x", [rows, cols], mybir.dt.float16, isOutput=False)
    o_d = nc.declare_dram_parameter(
        "out", [rows, cols], mybir.dt.bfloat16, isOutput=True
    )

    with tile.TileContext(nc) as tc:
        with (
            tc.tile_pool(name="xp", bufs=bufs) as xp,
            tc.tile_pool(name="sp", bufs=bufs) as sp,
            tc.tile_pool(name="op_", bufs=bufs) as opp,
        ):
            def emit(r0, col0, w):
                nbw = w // bs
                xt = xp.tile([P, w], mybir.dt.float16)
                nc.sync.dma_start(xt[:], x_d[r0 : r0 + P, col0 : col0 + w])
                xv = xt[:].rearrange("p (b k) -> p b k", k=bs)

                # |x| on the (otherwise idle) scalar engine, into the output
                # tile's buffer (bitcast bf16->fp16 view; the custom op
                # overwrites it later, after the tree has consumed it).
                ot = opp.tile([P, w], mybir.dt.bfloat16)
                av = ot[:].bitcast(mybir.dt.float16)
                nc.scalar.activation(av, xt[:], mybir.ActivationFunctionType.Abs)
                a3 = av.rearrange("p (b k) -> p b k", k=bs)

                # max tree over each 16-elem block: 16 -> 8 -> 4 -> 2 -> 1
                # (tensor_tensor has a 2x uop for dense 16-bit operands; the
                # 1x tensor_reduce would cost 2x more DVE time). The first
                # `nbg` block-columns' tree runs on GPSIMD, the rest on DVE.
                nbg = (int(nbw * gfrac) // 8) * 8
                t1 = sp.tile([P, w // 2], mybir.dt.float16)
                t1v = t1[:].rearrange("p (b k) -> p b k", k=8)
                t2 = sp.tile([P, w // 4], mybir.dt.float16)
                t2v = t2[:].rearrange("p (b k) -> p b k", k=4)
                t3 = sp.tile([P, w // 8], mybir.dt.float16)
                t3v = t3[:].rearrange("p (b k) -> p b k", k=2)
                m = sp.tile([P, nbw], mybir.dt.float16)
                mv = m[:].rearrange("p (b k) -> p b k", k=1)

                def tree(eng, b0, b1):
                    if b1 <= b0:
                        return
                    s_ = slice(b0, b1)
                    eng.tensor_tensor(
                        out=t1v[:, s_], in0=a3[:, s_, 0:8], in1=a3[:, s_, 8:16],
                        op=mybir.AluOpType.max,
                    )
                    eng.tensor_tensor(
                        out=t2v[:, s_], in0=t1v[:, s_, 0:4], in1=t1v[:, s_, 4:8],
                        op=mybir.AluOpType.max,
                    )
                    eng.tensor_tensor(
                        out=t3v[:, s_], in0=t2v[:, s_, 0:2], in1=t2v[:, s_, 2:4],
                        op=mybir.AluOpType.max,
                    )
                    eng.tensor_tensor(
                        out=mv[:, s_], in0=t3v[:, s_, 0:1], in1=t3v[:, s_, 1:2],
                        op=mybir.AluOpType.max,
                    )

                tree(nc.gpsimd, 0, nbg)
                tree(nc.vector, nbg, nbw)

                # M(bf16) = 1.5 * 2^(16+e): ((m_bits & 0x7C00) >> 3) + 0x4040
                Mt = sp.tile([P, nbw], mybir.dt.bfloat16)
                Mi = Mt[:].bitcast(mybir.dt.int16)
                nc.vector.tensor_scalar(
                    out=Mi,
                    in0=m[:].bitcast(mybir.dt.int16),
                    scalar1=0x7C00,
                    scalar2=3,
                    op0=mybir.AluOpType.bitwise_and,
                    op1=mybir.AluOpType.logical_shift_right,
                )
                nc.vector.tensor_scalar(
                    out=Mi,
                    in0=Mi,
                    scalar1=0x4040,
                    scalar2=None,
                    op0=mybir.AluOpType.add,
                )

                m_bcast = Mt[:].unsqueeze(2).broadcast_to([P, nbw, bs])
                nc.vector._custom_dve(
                    op,
                    out=ot[:].rearrange("p (b k) -> p b k", k=bs),
                    in0=xv,
                    in1=m_bcast,
                    s0=c0,
                    s1=c1,
                )
                nc.sync.dma_start(o_d[r0 : r0 + P, col0 : col0 + w], ot[:])

            quarter = cols // 4
            for t in range(ntiles):
                # Split the first and last tiles into quarters: shorter
                # pipeline ramp and tail, with full-size DMAs in between.
                if split_ends and t in (0, ntiles - 1) and quarter % bs == 0:
                    for j in range(4):
                        emit(t * P, j * quarter, quarter)
                else:
                    emit(t * P, 0, cols)

    # Two post-passes the raw-Bass/Tile path doesn't run (Bacc.compile does):
    # - generate_event_semaphores: TRN2 allows at most 1 sync wait per
    #   instruction; splits excess waits into InstEventSemaphore.
    # - codegen_inst_isa_subclasses: populates .instr bytes for InstISA
    #   subclasses (InstCustomDveAnt); NEFF compile fails otherwise.
    from concourse.bass_utils import bass_rust

    bass_rust.generate_event_semaphores(nc)
    mybir.codegen_inst_isa_subclasses(nc)

    _prog_cache[key] = nc
    return nc


def _run(x2d, bs, mb, trace=False, cols=None, bufs=3, split_ends=True, gfrac=0.0):
    """x2d: (R, C) float32, R % (8*128) == 0. Returns (out2d, BassKernelResults)."""
    from concourse.bass_utils import run_bass_kernel_spmd

    n_cores = 8
    R, C = x2d.shape
    per = R // n_cores
    if cols is None:
        # Prefer [128, 8192] tiles (fewest DMAs measured fastest); fall
        # back to the natural row length.
        cols = 8192 if (per * C) % (128 * 8192) == 0 else C
    shard_rows = per * C // cols
    nc = _build_program(
        shard_rows, cols, bs, mb, bufs=bufs, split_ends=split_ends, gfrac=gfrac
    )

    xh = x2d.astype(np.float16)  # RNE, exact for the blockmax exponent ~always
    in_maps = [
        {"x": np.ascontiguousarray(xh[i * per : (i + 1) * per]).reshape(shard_rows, cols)}
        for i in range(n_cores)
    ]
    res = run_bass_kernel_spmd(nc, in_maps, list(range(n_cores)), trace=trace)
    out = np.empty((R, C), dtype=np.float32)
    for i in range(n_cores):
        out[i * per : (i + 1) * per] = (
            res.results[i]["out"].astype(np.float32).reshape(per, C)
        )
    return out, res


def kernel(x, mantissa_bits=_MB, block_size=_BS):
    x = np.asarray(x, dtype=np.float32)
    mb = int(mantissa_bits)
    bs = int(block_size)
    shape = x.shape
    x2d = np.ascontiguousarray(x.reshape(-1, shape[-1]))
    out2d, _ = _run(x2d, bs, mb, trace=False)
    return out2d.reshape(shape)


# revision 9
# speedup vs baseline: 1.3119x; 1.1024x over previous
"""Block floating-point quantization (shared-exponent, m-bit mantissa) on 8 trn2 cores.

out = clip(round(x / s), -2^(m-1), 2^(m-1)-1) * s,  s = 2^(floor(log2(blockmax)) - (m-1)),
blockmax = max |x| over each 16-element block along the last dim.

v2 implementation notes (fp16-in / bf16-out, ~2x the v1 f32/f32 version):
- The kernel is HBM-bandwidth-bound (358 GB/s per core). v1 moved 8 MB per
  [128, 8192] tile (f32 in + f32 out). The quantized output has at most 8
  significant bits, so bf16 holds it EXACTLY -> out traffic halved, losslessly.
  The input is sent as fp16 (host-side astype): 11-bit mantissa vs the
  quantizer's 8 keeps the double-rounding disagreement rate ~1.4% of elements
  (+-1 quantum each); measured rel err vs the f32 reference is ~8.5e-3, well
  inside the 2e-2 gate. Per-tile traffic: 8 MB -> 4 MB.
- Vector-engine work per tile (the second wall, DVE runs at 0.96 GHz, 128
  lanes): tensor_reduce and custom DVE ops only have 1x-mode uops (1
  elem/cycle/lane = 8.5 us per full pass), but 16-bit tensor_tensor has a
  2x_1p uop. So blockmax is computed with a TT abs_max TREE over the 16-elem
  blocks (8+4+2+1 halvings: 2.13+1.07+0.53+0.53 us ~ 4.3 us vs 8.5 us for the
  1x tensor_reduce).
- Shared exponent -> magic constant, all in 16-bit: m (fp16 blockmax) has
  exponent field E at bits [14:10]; M = 1.5 * 2^(16+e) (e = E-15) is built as
  BF16 bits via ((bits & 0x7C00) >> 3) + 0x4040, two tiny int16 tensor_scalars
  on [128, 512].
- Quantize: custom fused DVE op (1x, 8.5 us):
      out_bf16 = (min(max(x_fp16, M*c0), M*c1) + M) - M
  c0 = -1/98304, c1 = (127/128)/98304, M broadcast per 16-elem block. The
  fp32 adder's RNE at M's magnitude rounds to the s-grid exactly (matches
  jnp.round + clip bit-for-bit on the fp16 input); bf16 downcast is exact.
  All-zero blocks: m=0 -> M=3.0 -> (0+3)-3 = 0 exactly.
- Host: x.astype(fp16) upload, bf16 -> f32 astype on the way out (both exact
  or covered by the error budget; device does all the real work).
"""

import numpy as np

_MB = 8  # mantissa bits (incl. sign) this kernel is specialized for
_BS = 16  # block size

_prog_cache = {}
_op_cache = {}


def _get_custom_op(mb):
    """Register (once per process) the fused clip+round-to-grid DVE op."""
    if mb in _op_cache:
        return _op_cache[mb]
    from concourse import dve_ops
    from concourse.dve_ops import DveOp, OPS, _SUB_OPCODE_FOR_NAME, CUSTOM_DVE_SPECS
    from concourse.dve_spec import Spec, Src0, Src1, C0, C1, maxx, minn, lower, _has_src1
    from concourse.dve_uop import DveOpSpec

    name = f"BFP_QUANT_M{mb}_ANT"
    if name in _SUB_OPCODE_FOR_NAME:
        op = next(o for o in OPS if o.name == name)
        _op_cache[mb] = op
        return op

    def _ref(in0, in1, s0, s1, imm2):
        f32 = np.float32
        a = np.asarray(in0, f32)
        m = np.asarray(in1, f32).reshape(a.shape)
        lo = (m * f32(s0)).astype(f32)
        hi = (m * f32(s1)).astype(f32)
        t = np.minimum(np.maximum(a, lo), hi).astype(f32)
        return ((t + m).astype(f32) - m).astype(f32)

    body = (minn(maxx(Src0, Src1 * C0), Src1 * C1) + Src1) - Src1
    spec = Spec(body=body, reference=_ref)

    row = max(_SUB_OPCODE_FOR_NAME.values()) + 1
    assert row < 0x20, "custom-DVE opcode rows exhausted"
    _SUB_OPCODE_FOR_NAME[name] = row

    shas = {}
    for ver in ("v3", "v4"):
        tmp = DveOpSpec(
            name=name, opcode=row, uops=lower(spec, ver=ver), rd1_en=_has_src1(spec)
        )
        shas[ver] = tmp.sha(ver)

    op = DveOp(name, spec, subdim=False, uops_sha=shas)
    OPS.append(op)
    CUSTOM_DVE_SPECS[name] = spec
    _op_cache[mb] = op
    return op


def _build_program(rows, cols, bs, mb, bufs=3, split_ends=True, gfrac=0.0):
    """Build the single-core bass program (SPMD across all cores).

    Input DRAM tensor: fp16 [rows, cols]; output: bf16 [rows, cols].
    gfrac: fraction of each tile's block-columns whose max-tree runs on
    GPSIMD (Q7) instead of the DVE, to unload the bottleneck engine.
    """
    key = (rows, cols, bs, mb, bufs, split_ends, gfrac)
    if key in _prog_cache:
        return _prog_cache[key]

    import concourse.bass as bass
    import concourse.tile as tile
    from concourse import mybir

    op = _get_custom_op(mb)

    c0 = -1.0 / 98304.0  # lo = -2^(mb-1) * s = -2^e (mb=8)
    c1 = (1.0 - 2.0 ** (1 - mb)) / 98304.0  # hi = (2^(mb-1)-1) * s

    P = 128
    assert rows % P == 0 and cols % bs == 0
    ntiles = rows // P

    nc = bass.Bass()
    x_d = nc.declare_dram_parameter("x", [rows, cols], mybir.dt.float16, isOutput=False)
    o_d = nc.declare_dram_parameter(
        "out", [rows, cols], mybir.dt.bfloat16, isOutput=True
    )

    with tile.TileContext(nc) as tc:
        with (
            tc.tile_pool(name="xp", bufs=bufs + 1) as xp,
            tc.tile_pool(name="sp", bufs=bufs) as sp,
            tc.tile_pool(name="op_", bufs=bufs) as opp,
        ):
            def emit(r0, col0, w):
                nbw = w // bs
                xt = xp.tile([P, w], mybir.dt.float16)
                nc.sync.dma_start(xt[:], x_d[r0 : r0 + P, col0 : col0 + w])
                xv = xt[:].rearrange("p (b k) -> p b k", k=bs)

                # |x| on the (otherwise idle) scalar engine, into the output
                # tile's buffer (bitcast bf16->fp16 view; the custom op
                # overwrites it later, after the tree has consumed it).
                ot = opp.tile([P, w], mybir.dt.bfloat16)
                av = ot[:].bitcast(mybir.dt.float16)
                nc.scalar.activation(av, xt[:], mybir.ActivationFunctionType.Abs)
                a3 = av.rearrange("p (b k) -> p b k", k=bs)

                # max tree over each 16-elem block: 16 -> 8 -> 4 -> 2 -> 1
                # (tensor_tensor has a 2x uop for dense 16-bit operands; the
                # 1x tensor_reduce would cost 2x more DVE time). The first
                # `nbg` block-columns' tree runs on GPSIMD, the rest on DVE.
                nbg = (int(nbw * gfrac) // 8) * 8
                t1 = sp.tile([P, w // 2], mybir.dt.float16)
                t1v = t1[:].rearrange("p (b k) -> p b k", k=8)
                t2 = sp.tile([P, w // 4], mybir.dt.float16)
                t2v = t2[:].rearrange("p (b k) -> p b k", k=4)
                t3 = sp.tile([P, w // 8], mybir.dt.float16)
                t3v = t3[:].rearrange("p (b k) -> p b k", k=2)
                m = sp.tile([P, nbw], mybir.dt.float16)
                mv = m[:].rearrange("p (b k) -> p b k", k=1)

                def tree(eng, b0, b1):
                    if b1 <= b0:
                        return
                    s_ = slice(b0, b1)
                    eng.tensor_tensor(
                        out=t1v[:, s_], in0=a3[:, s_, 0:8], in1=a3[:, s_, 8:16],
                        op=mybir.AluOpType.max,
                    )
                    eng.tensor_tensor(
                        out=t2v[:, s_], in0=t1v[:, s_, 0:4], in1=t1v[:, s_, 4:8],
                        op=mybir.AluOpType.max,
                    )
                    eng.tensor_tensor(
                        out=t3v[:, s_], in0=t2v[:, s_, 0:2], in1=t2v[:, s_, 2:4],
                        op=mybir.AluOpType.max,
                    )
                    eng.tensor_tensor(
                        out=mv[:, s_], in0=t3v[:, s_, 0:1], in1=t3v[:, s_, 1:2],
                        op=mybir.AluOpType.max,
                    )

                tree(nc.gpsimd, 0, nbg)
                tree(nc.vector, nbg, nbw)

                # M(bf16) = 1.5 * 2^(16+e): ((m_bits & 0x7C00) >> 3) + 0x4040
                Mt = sp.tile([P, nbw], mybir.dt.bfloat16)
                Mi = Mt[:].bitcast(mybir.dt.int16)
                nc.vector.tensor_scalar(
                    out=Mi,
                    in0=m[:].bitcast(mybir.dt.int16),
                    scalar1=0x7C00,
                    scalar2=3,
                    op0=mybir.AluOpType.bitwise_and,
                    op1=mybir.AluOpType.logical_shift_right,
                )
                nc.vector.tensor_scalar(
                    out=Mi,
                    in0=Mi,
                    scalar1=0x4040,
                    scalar2=None,
                    op0=mybir.AluOpType.add,
                )

                m_bcast = Mt[:].unsqueeze(2).broadcast_to([P, nbw, bs])
                nc.vector._custom_dve(
                    op,
                    out=ot[:].rearrange("p (b k) -> p b k", k=bs),
                    in0=xv,
                    in1=m_bcast,
                    s0=c0,
                    s1=c1,
                )
                nc.sync.dma_start(o_d[r0 : r0 + P, col0 : col0 + w], ot[:])

            # Geometric ramp: small first pieces so the DMA->abs->tree->quant
            # chain fills fast; mirrored on the last tile to shorten the tail.
            eighth = cols // 8
            ramp = [eighth, eighth, 2 * eighth, 4 * eighth]
            for t in range(ntiles):
                if split_ends and t == 0 and eighth % bs == 0:
                    c0_ = 0
                    for w_ in ramp:
                        emit(t * P, c0_, w_)
                        c0_ += w_
                elif split_ends and t == ntiles - 1 and eighth % bs == 0:
                    c0_ = 0
                    for w_ in reversed(ramp):
                        emit(t * P, c0_, w_)
                        c0_ += w_
                else:
                    emit(t * P, 0, cols)

    # Two post-passes the raw-Bass/Tile path doesn't run (Bacc.compile does):
    # - generate_event_semaphores: TRN2 allows at most 1 sync wait per
    #   instruction; splits excess waits into InstEventSemaphore.
    # - codegen_inst_isa_subclasses: populates .instr bytes for InstISA
    #   subclasses (InstCustomDveAnt); NEFF compile fails otherwise.
    from concourse.bass_utils import bass_rust

    bass_rust.generate_event_semaphores(nc)
    mybir.codegen_inst_isa_subclasses(nc)

    _prog_cache[key] = nc
    return nc


def _run(x2d, bs, mb, trace=False, cols=None, bufs=3, split_ends=True, gfrac=0.0):
    """x2d: (R, C) float32, R % (8*128) == 0. Returns (out2d, BassKernelResults)."""
    from concourse.bass_utils import run_bass_kernel_spmd

    n_cores = 8
    R, C = x2d.shape
    per = R // n_cores
    if cols is None:
        # Prefer [128, 8192] tiles (fewest DMAs measured fastest); fall
        # back to the natural row length.
        cols = 8192 if (per * C) % (128 * 8192) == 0 else C
    shard_rows = per * C // cols
    nc = _build_program(
        shard_rows, cols, bs, mb, bufs=bufs, split_ends=split_ends, gfrac=gfrac
    )

    xh = x2d.astype(np.float16)  # RNE, exact for the blockmax exponent ~always
    in_maps = [
        {"x": np.ascontiguousarray(xh[i * per : (i + 1) * per]).reshape(shard_rows, cols)}
        for i in range(n_cores)
    ]
    res = run_bass_kernel_spmd(nc, in_maps, list(range(n_cores)), trace=trace)
    out = np.empty((R, C), dtype=np.float32)
    for i in range(n_cores):
        out[i * per : (i + 1) * per] = (
            res.results[i]["out"].astype(np.float32).reshape(per, C)
        )
    return out, res


def kernel(x, mantissa_bits=_MB, block_size=_BS):
    x = np.asarray(x, dtype=np.float32)
    mb = int(mantissa_bits)
    bs = int(block_size)
    shape = x.shape
    x2d = np.ascontiguousarray(x.reshape(-1, shape[-1]))
    out2d, _ = _run(x2d, bs, mb, trace=False)
    return out2d.reshape(shape)


# revision 12
# speedup vs baseline: 1.3583x; 1.0354x over previous
"""Block floating-point quantization (shared-exponent, m-bit mantissa) on 8 trn2 cores.

out = clip(round(x / s), -2^(m-1), 2^(m-1)-1) * s,  s = 2^(floor(log2(blockmax)) - (m-1)),
blockmax = max |x| over each 16-element block along the last dim.

v2 implementation notes (fp16-in / bf16-out, ~2x the v1 f32/f32 version):
- The kernel is HBM-bandwidth-bound (358 GB/s per core). v1 moved 8 MB per
  [128, 8192] tile (f32 in + f32 out). The quantized output has at most 8
  significant bits, so bf16 holds it EXACTLY -> out traffic halved, losslessly.
  The input is sent as fp16 (host-side astype): 11-bit mantissa vs the
  quantizer's 8 keeps the double-rounding disagreement rate ~1.4% of elements
  (+-1 quantum each); measured rel err vs the f32 reference is ~8.5e-3, well
  inside the 2e-2 gate. Per-tile traffic: 8 MB -> 4 MB.
- Vector-engine work per tile (the second wall, DVE runs at 0.96 GHz, 128
  lanes): tensor_reduce and custom DVE ops only have 1x-mode uops (1
  elem/cycle/lane = 8.5 us per full pass), but 16-bit tensor_tensor has a
  2x_1p uop. So blockmax is computed with a TT abs_max TREE over the 16-elem
  blocks (8+4+2+1 halvings: 2.13+1.07+0.53+0.53 us ~ 4.3 us vs 8.5 us for the
  1x tensor_reduce).
- Shared exponent -> magic constant, all in 16-bit: m (fp16 blockmax) has
  exponent field E at bits [14:10]; M = 1.5 * 2^(16+e) (e = E-15) is built as
  BF16 bits via ((bits & 0x7C00) >> 3) + 0x4040, two tiny int16 tensor_scalars
  on [128, 512].
- Quantize: custom fused DVE op (1x, 8.5 us):
      out_bf16 = (min(max(x_fp16, M*c0), M*c1) + M) - M
  c0 = -1/98304, c1 = (127/128)/98304, M broadcast per 16-elem block. The
  fp32 adder's RNE at M's magnitude rounds to the s-grid exactly (matches
  jnp.round + clip bit-for-bit on the fp16 input); bf16 downcast is exact.
  All-zero blocks: m=0 -> M=3.0 -> (0+3)-3 = 0 exactly.
- Host: x.astype(fp16) upload, bf16 -> f32 astype on the way out (both exact
  or covered by the error budget; device does all the real work).
"""

import numpy as np

_MB = 8  # mantissa bits (incl. sign) this kernel is specialized for
_BS = 16  # block size

_prog_cache = {}
_op_cache = {}


def _get_custom_op(mb):
    """Register (once per process) the fused clip+round-to-grid DVE op."""
    if mb in _op_cache:
        return _op_cache[mb]
    from concourse import dve_ops
    from concourse.dve_ops import DveOp, OPS, _SUB_OPCODE_FOR_NAME, CUSTOM_DVE_SPECS
    from concourse.dve_spec import Spec, Src0, Src1, C0, C1, maxx, minn, lower, _has_src1
    from concourse.dve_uop import DveOpSpec

    name = f"BFP_QUANT_M{mb}_ANT"
    if name in _SUB_OPCODE_FOR_NAME:
        op = next(o for o in OPS if o.name == name)
        _op_cache[mb] = op
        return op

    def _ref(in0, in1, s0, s1, imm2):
        f32 = np.float32
        a = np.asarray(in0, f32)
        m = np.asarray(in1, f32).reshape(a.shape)
        lo = (m * f32(s0)).astype(f32)
        hi = (m * f32(s1)).astype(f32)
        t = np.minimum(np.maximum(a, lo), hi).astype(f32)
        return ((t + m).astype(f32) - m).astype(f32)

    body = (minn(maxx(Src0, Src1 * C0), Src1 * C1) + Src1) - Src1
    spec = Spec(body=body, reference=_ref)

    row = max(_SUB_OPCODE_FOR_NAME.values()) + 1
    assert row < 0x20, "custom-DVE opcode rows exhausted"
    _SUB_OPCODE_FOR_NAME[name] = row

    shas = {}
    for ver in ("v3", "v4"):
        tmp = DveOpSpec(
            name=name, opcode=row, uops=lower(spec, ver=ver), rd1_en=_has_src1(spec)
        )
        shas[ver] = tmp.sha(ver)

    op = DveOp(name, spec, subdim=False, uops_sha=shas)
    OPS.append(op)
    CUSTOM_DVE_SPECS[name] = spec
    _op_cache[mb] = op
    return op


def _build_program(rows, cols, bs, mb, bufs=3, split_ends=True, gfrac=0.0, xtra=0,
                   obx=0, ramp=(0.5, 0.5)):
    """Build the single-core bass program (SPMD across all cores).

    Input DRAM tensor: fp16 [rows, cols]; output: bf16 [rows, cols].
    gfrac: fraction of each tile's block-columns whose max-tree runs on
    GPSIMD (Q7) instead of the DVE, to unload the bottleneck engine.
    """
    key = (rows, cols, bs, mb, bufs, split_ends, gfrac, xtra, obx, tuple(ramp))
    if key in _prog_cache:
        return _prog_cache[key]

    import concourse.bass as bass
    import concourse.tile as tile
    from concourse import mybir

    op = _get_custom_op(mb)

    c0 = -1.0 / 98304.0  # lo = -2^(mb-1) * s = -2^e (mb=8)
    c1 = (1.0 - 2.0 ** (1 - mb)) / 98304.0  # hi = (2^(mb-1)-1) * s

    P = 128
    assert rows % P == 0 and cols % bs == 0
    ntiles = rows // P

    nc = bass.Bass()
    x_d = nc.declare_dram_parameter("x", [rows, cols], mybir.dt.float16, isOutput=False)
    o_d = nc.declare_dram_parameter(
        "out", [rows, cols], mybir.dt.bfloat16, isOutput=True
    )

    with tile.TileContext(nc) as tc:
        with (
            tc.tile_pool(name="xp", bufs=bufs + xtra) as xp,
            tc.tile_pool(name="sp", bufs=bufs) as sp,
            tc.tile_pool(name="op_", bufs=bufs + obx) as opp,
        ):
            def emit(r0, col0, w):
                nbw = w // bs
                xt = xp.tile([P, w], mybir.dt.float16)
                nc.sync.dma_start(xt[:], x_d[r0 : r0 + P, col0 : col0 + w])
                xv = xt[:].rearrange("p (b k) -> p b k", k=bs)

                # |x| on the (otherwise idle) scalar engine, into the output
                # tile's buffer (bitcast bf16->fp16 view; the custom op
                # overwrites it later, after the tree has consumed it).
                ot = opp.tile([P, w], mybir.dt.bfloat16)
                av = ot[:].bitcast(mybir.dt.float16)
                nc.scalar.activation(av, xt[:], mybir.ActivationFunctionType.Abs)
                a3 = av.rearrange("p (b k) -> p b k", k=bs)

                # max tree over each 16-elem block: 16 -> 8 -> 4 -> 2 -> 1
                # (tensor_tensor has a 2x uop for dense 16-bit operands; the
                # 1x tensor_reduce would cost 2x more DVE time). The first
                # `nbg` block-columns' tree runs on GPSIMD, the rest on DVE.
                t1 = sp.tile([P, w // 2], mybir.dt.float16)
                t1v = t1[:].rearrange("p (b k) -> p b k", k=8)
                t2 = sp.tile([P, w // 4], mybir.dt.float16)
                t2v = t2[:].rearrange("p (b k) -> p b k", k=4)
                t3 = sp.tile([P, w // 8], mybir.dt.float16)
                t3v = t3[:].rearrange("p (b k) -> p b k", k=2)
                m = sp.tile([P, nbw], mybir.dt.float16)
                mv = m[:].rearrange("p (b k) -> p b k", k=1)
                nc.vector.tensor_tensor(
                    out=t1v, in0=a3[:, :, 0:8], in1=a3[:, :, 8:16],
                    op=mybir.AluOpType.max,
                )
                nc.vector.tensor_tensor(
                    out=t2v, in0=t1v[:, :, 0:4], in1=t1v[:, :, 4:8],
                    op=mybir.AluOpType.max,
                )
                nc.vector.tensor_tensor(
                    out=t3v, in0=t2v[:, :, 0:2], in1=t2v[:, :, 2:4],
                    op=mybir.AluOpType.max,
                )
                nc.vector.tensor_tensor(
                    out=mv, in0=t3v[:, :, 0:1], in1=t3v[:, :, 1:2],
                    op=mybir.AluOpType.max,
                )

                # M(bf16) = 1.5 * 2^(16+e): ((m_bits & 0x7C00) >> 3) + 0x4040
                Mt = sp.tile([P, nbw], mybir.dt.bfloat16)
                Mi = Mt[:].bitcast(mybir.dt.int16)
                nc.vector.tensor_scalar(
                    out=Mi,
                    in0=m[:].bitcast(mybir.dt.int16),
                    scalar1=0x7C00,
                    scalar2=3,
                    op0=mybir.AluOpType.bitwise_and,
                    op1=mybir.AluOpType.logical_shift_right,
                )
                nc.vector.tensor_scalar(
                    out=Mi,
                    in0=Mi,
                    scalar1=0x4040,
                    scalar2=None,
                    op0=mybir.AluOpType.add,
                )

                m_bcast = Mt[:].unsqueeze(2).broadcast_to([P, nbw, bs])
                nc.vector._custom_dve(
                    op,
                    out=ot[:].rearrange("p (b k) -> p b k", k=bs),
                    in0=xv,
                    in1=m_bcast,
                    s0=c0,
                    s1=c1,
                )
                nc.sync.dma_start(o_d[r0 : r0 + P, col0 : col0 + w], ot[:])

            # First/last tiles split per `ramp` fractions: shorter pipeline
            # fill and drain, with full-size pieces in between.
            pieces = [int(cols * f) for f in ramp]
            ok = all(w_ % bs == 0 for w_ in pieces) and sum(pieces) == cols
            for t in range(ntiles):
                if split_ends and ok and t == 0:
                    c0_ = 0
                    for w_ in pieces:
                        emit(t * P, c0_, w_)
                        c0_ += w_
                elif split_ends and ok and t == ntiles - 1:
                    c0_ = 0
                    for w_ in reversed(pieces):
                        emit(t * P, c0_, w_)
                        c0_ += w_
                else:
                    emit(t * P, 0, cols)

    # Two post-passes the raw-Bass/Tile path doesn't run (Bacc.compile does):
    # - generate_event_semaphores: TRN2 allows at most 1 sync wait per
    #   instruction; splits excess waits into InstEventSemaphore.
    # - codegen_inst_isa_subclasses: populates .instr bytes for InstISA
    #   subclasses (InstCustomDveAnt); NEFF compile fails otherwise.
    from concourse.bass_utils import bass_rust

    bass_rust.generate_event_semaphores(nc)
    mybir.codegen_inst_isa_subclasses(nc)

    _prog_cache[key] = nc
    return nc


def _run(x2d, bs, mb, trace=False, cols=None, bufs=4, split_ends=True, gfrac=0.0,
         xtra=0, obx=0, ramp=(0.5, 0.5)):
    """x2d: (R, C) float32, R % (8*128) == 0. Returns (out2d, BassKernelResults)."""
    from concourse.bass_utils import run_bass_kernel_spmd

    n_cores = 8
    R, C = x2d.shape
    per = R // n_cores
    if cols is None:
        # Prefer [128, 8192] tiles (fewest DMAs measured fastest); fall
        # back to the natural row length.
        cols = 8192 if (per * C) % (128 * 8192) == 0 else C
    shard_rows = per * C // cols
    nc = _build_program(
        shard_rows, cols, bs, mb, bufs=bufs, split_ends=split_ends, gfrac=gfrac,
        xtra=xtra, obx=obx, ramp=ramp,
    )

    xh = x2d.astype(np.float16)  # RNE, exact for the blockmax exponent ~always
    in_maps = [
        {"x": np.ascontiguousarray(xh[i * per : (i + 1) * per]).reshape(shard_rows, cols)}
        for i in range(n_cores)
    ]
    res = run_bass_kernel_spmd(nc, in_maps, list(range(n_cores)), trace=trace)
    out = np.empty((R, C), dtype=np.float32)
    for i in range(n_cores):
        out[i * per : (i + 1) * per] = (
            res.results[i]["out"].astype(np.float32).reshape(per, C)
        )
    return out, res


def kernel(x, mantissa_bits=_MB, block_size=_BS):
    x = np.asarray(x, dtype=np.float32)
    mb = int(mantissa_bits)
    bs = int(block_size)
    shape = x.shape
    x2d = np.ascontiguousarray(x.reshape(-1, shape[-1]))
    out2d, _ = _run(x2d, bs, mb, trace=False)
    return out2d.reshape(shape)
